# revision 1
# baseline (speedup 1.0000x reference)
"""Trainium2 Bass kernel for nn_CompleteNetwork (gnn_message_passing).

Computes, on 8 NeuronCores:
  - 3-layer GNN over two 6000-atom proteins (matmuls + neighbor gather-means)
  - per-residue means -> r1, r2 [300, 512]
  - FC stack 1024->128->64->1 (no nonlinearity -> collapsed on host into a
    single 1024->1 linear map, exact algebra)
  - log_softmax over axis=1 (size 1), emitted exactly as h - h.

Sharding: atoms are sharded 8 ways (750 real + 18 pad rows per core).  After
each of layers 1 and 2 the per-core activation shard is AllGathered into a
full bf16 row table in device DRAM, which feeds the next layer's neighbor
gathers (dma_gather transpose mode: gathered rows land as [F_in, atoms]
columns, directly consumable as matmul rhs).  The neighbor-sum over K=10 is
performed by PSUM accumulation on the tensor engine (the stationary weight is
reused across the 10 slots).  Per-atom 1/count scaling is applied with
pre-broadcast scale tiles on the vector engine.  Residue sums use a 0/1
membership-matrix matmul followed by a small AllReduce.
"""

import numpy as np
import ml_dtypes

bf16 = ml_dtypes.bfloat16

# ---- problem dims (hardcoded; harness provides full inputs matching these) --
N = 6000          # atoms per protein
NRES = 300        # residues
KNEI = 10         # neighbors per atom
NA, NR = 12, 23   # atom / residue categories
F0 = 128          # layer-0 feature width (35 used, padded)
F1, F2, F3 = 128, 256, 512
NCORES = 8
SH_REAL = 750     # real atoms per core
SH = 768          # padded shard
NTOT = SH * NCORES   # 6144
ZROW = 750        # guaranteed-zero row in new numbering (core0 pad)
NIDX = KNEI * SH  # 7680 gather indices per (protein, table)
RPAD = 384        # residues padded to 3 partition tiles
RT = 3
NPAIR = NRES * NRES  # 90000

_PROG = None  # cached compiled program


# ===========================================================================
# device program
# ===========================================================================
def _build_program():
    import concourse.mybir as mybir
    import concourse.tile as tile
    from concourse import bacc, library_config
    from concourse.masks import make_identity

    dt = mybir.dt
    nc = bacc.Bacc("TRN2", target_bir_lowering=False, num_swdge_queues=4)

    # ---- external inputs -------------------------------------------------
    t0 = [nc.dram_tensor(f"t0_{p}", [NTOT, F0], dt.bfloat16, kind="ExternalInput")
          for p in range(2)]
    z0t = [nc.dram_tensor(f"z0t_{p}", [128, SH], dt.bfloat16, kind="ExternalInput")
           for p in range(2)]
    idx_in = {(p, t): nc.dram_tensor(f"idx_{p}{t}", [128, NIDX // 16], dt.int16,
                                     kind="ExternalInput")
              for p in range(2) for t in range(2)}
    scl_in = nc.dram_tensor("scl", [128, 4, SH], dt.bfloat16, kind="ExternalInput")
    w0v = nc.dram_tensor("w0v", [128, F1], dt.bfloat16, kind="ExternalInput")
    w1s = nc.dram_tensor("w1s", [128, F1], dt.bfloat16, kind="ExternalInput")
    w1d = nc.dram_tensor("w1d", [128, F1], dt.bfloat16, kind="ExternalInput")
    w2v = nc.dram_tensor("w2v", [128, F2], dt.bfloat16, kind="ExternalInput")
    w2s = nc.dram_tensor("w2s", [128, F2], dt.bfloat16, kind="ExternalInput")
    w2d = nc.dram_tensor("w2d", [128, F2], dt.bfloat16, kind="ExternalInput")
    w3v = nc.dram_tensor("w3v", [128, 2, F3], dt.bfloat16, kind="ExternalInput")
    w3s = nc.dram_tensor("w3s", [128, 2, F3], dt.bfloat16, kind="ExternalInput")
    w3d = nc.dram_tensor("w3d", [128, 2, F3], dt.bfloat16, kind="ExternalInput")
    s_in = [nc.dram_tensor(f"s_{p}", [128, 6, RPAD], dt.bfloat16, kind="ExternalInput")
            for p in range(2)]
    ricnt_in = nc.dram_tensor("ricnt", [128, RT], dt.float32, kind="ExternalInput")
    wab_in = nc.dram_tensor("wab", [128, 2, F3], dt.float32, kind="ExternalInput")
    beff_in = nc.dram_tensor("beff", [128, 1], dt.float32, kind="ExternalInput")

    out_d = nc.dram_tensor("out", [NPAIR, 1], dt.float32, kind="ExternalOutput")
    hdbg_d = nc.dram_tensor("hdbg", [RPAD, RPAD], dt.float32, kind="ExternalOutput")

    FIN = {1: F0, 2: F1, 3: F2}     # gather/self input width per layer
    FOUT = {1: F1, 2: F2, 3: F3}
    KCH = {1: 1, 2: 1, 3: 2}        # 128-row contraction chunks
    MT = {1: 1, 2: 2, 3: 4}         # 128-col output tiles
    NCHUNK = 384                    # atom columns per psum tile
    NNC = SH // NCHUNK              # 2

    with tile.TileContext(nc) as tc:
        with (
            tc.tile_pool(name="const", bufs=1) as cpool,
            tc.tile_pool(name="g", bufs=16) as gpool,
            tc.tile_pool(name="zt", bufs=2) as ztpool,
            tc.tile_pool(name="zrow", bufs=2) as zrpool,
            tc.tile_pool(name="tmp", bufs=2) as tpool,
            tc.tile_pool(name="ks", bufs=3) as kspool,
            tc.tile_pool(name="rh", bufs=1) as rhpool,
            tc.tile_pool(name="acc", bufs=6, space="PSUM") as apool,
            tc.tile_pool(name="tr", bufs=2, space="PSUM") as trpool,
            tc.tile_pool(name="dram", bufs=1, space="DRAM") as dram,
        ):
            nc.gpsimd.load_library(library_config.mlp)

            # ---- constant loads -----------------------------------------
            def load_const(name, src, shape, dtype):
                t = cpool.tile(shape, dtype, tag=name, name=name)
                nc.sync.dma_start(t[:], src[:])
                return t

            ident = cpool.tile([128, 128], dt.bfloat16, tag="ident")
            make_identity(nc, ident[:])
            ident32 = cpool.tile([128, 128], dt.float32, tag="ident32")
            make_identity(nc, ident32[:])
            ones1 = cpool.tile([1, 128], dt.float32, tag="ones1")
            nc.gpsimd.memset(ones1[:], 1.0)

            z0t_sb = [load_const(f"z0t{p}", z0t[p], [128, SH], dt.bfloat16)
                      for p in range(2)]
            idx_sb = {k: load_const(f"idx{k}", v, [128, NIDX // 16], dt.int16)
                      for k, v in idx_in.items()}
            scl_sb = load_const("scl", scl_in, [128, 4, SH], dt.bfloat16)
            wsb = {}
            for nm, src, shp in [
                ("w0v", w0v, [128, F1]), ("w1s", w1s, [128, F1]), ("w1d", w1d, [128, F1]),
                ("w2v", w2v, [128, F2]), ("w2s", w2s, [128, F2]), ("w2d", w2d, [128, F2]),
                ("w3v", w3v, [128, 2, F3]), ("w3s", w3s, [128, 2, F3]),
                ("w3d", w3d, [128, 2, F3]),
            ]:
                wsb[nm] = load_const(nm, src, shp, dt.bfloat16)
            s_sb = [load_const(f"s{p}", s_in[p], [128, 6, RPAD], dt.bfloat16)
                    for p in range(2)]
            ricnt = load_const("ricnt", ricnt_in, [128, RT], dt.float32)
            wab = load_const("wab", wab_in, [128, 2, F3], dt.float32)
            beff = load_const("beff", beff_in, [128, 1], dt.float32)

            # ---- DRAM tables + bounce buffers ---------------------------
            tabs = {1: [dram.tile([NTOT, F1], dt.bfloat16, tag=f"t1_{p}", name=f"t1_{p}", addr_space="Shared") for p in range(2)],
                    2: [dram.tile([NTOT, F2], dt.bfloat16, tag=f"t2_{p}", name=f"t2_{p}", addr_space="Shared") for p in range(2)]}
            zbounce = {(lay, p): dram.tile([SH, FOUT[lay]], dt.bfloat16, tag=f"zb{lay}{p}", name=f"zb{lay}{p}")
                       for lay in (1, 2) for p in range(2)}
            rbounce_in = [dram.tile([RPAD, F3], dt.bfloat16, tag=f"rbi{p}", name=f"rbi{p}") for p in range(2)]
            rbounce_out = [dram.tile([RPAD, F3], dt.bfloat16, tag=f"rbo{p}", name=f"rbo{p}", addr_space="Shared") for p in range(2)]

            rg = [list(range(NCORES))]
            qrr = [0]  # gather queue round-robin

            def scl_ap(p, t, nch):
                return scl_sb[:, p * 2 + t, nch * NCHUNK:(nch + 1) * NCHUNK]

            # weight lhsT slice [128, 128] for (layer, term, kc, mt)
            def w_ap(lay, term, kc, mt):
                nm = {1: {"v": "w0v", "s": "w1s", "d": "w1d"},
                      2: {"v": "w2v", "s": "w2s", "d": "w2d"},
                      3: {"v": "w3v", "s": "w3s", "d": "w3d"}}[lay][term]
                w = wsb[nm]
                if lay == 3:
                    return w[:, kc, mt * 128:(mt + 1) * 128]
                return w[:, mt * 128:(mt + 1) * 128]

            zT_prev = [None, None]   # per protein: current zT sbuf tile
            z3row = [None, None]

            # ================= GNN layers ================================
            for lay in (1, 2, 3):
                fin, fout, kch, mt_n = FIN[lay], FOUT[lay], KCH[lay], MT[lay]
                for p in range(2):
                    src = t0[p] if lay == 1 else tabs[lay - 1][p]
                    # ---- neighbor gathers (transpose mode, <=896 idx/inst)
                    # G[t]: [128, KNEI, kch, SH] bf16; [:, k, :, j] = row idx
                    # gathers per (table, k-slot); DVE K-sum chases them
                    GS = {}
                    for t in range(2):
                        acc = kspool.tile([128, kch, SH], dt.bfloat16,
                                          tag="ks", name="ksacc")
                        gprev = None
                        for k in range(KNEI):
                            gk = gpool.tile([128, kch, SH], dt.bfloat16,
                                            tag="g", name="gk")
                            isl = idx_sb[(p, t)][:, k * (SH // 16):(k + 1) * (SH // 16)]
                            nc.gpsimd.dma_gather(
                                gk[:], src[:], isl, SH, SH, fin,
                                transpose=True, queue_num=(qrr[0] % 4))
                            qrr[0] += 1
                            if k == 1:
                                nc.vector.tensor_tensor(
                                    acc[:], gprev[:], gk[:], mybir.AluOpType.add)
                            elif k > 1:
                                nc.vector.tensor_tensor(
                                    acc[:], acc[:], gk[:], mybir.AluOpType.add)
                            gprev = gk
                        GS[t] = acc[:]

                    zT = ztpool.tile([128, mt_n, SH], dt.bfloat16, tag=f"z{lay}t")
                    selfT = z0t_sb[p] if lay == 1 else zT_prev[p]

                    for mt in range(mt_n):
                        for nch in range(NNC):
                            nsl = slice(nch * NCHUNK, (nch + 1) * NCHUNK)
                            ps_v = apool.tile([128, NCHUNK], dt.float32, tag="acc")
                            ps_s = apool.tile([128, NCHUNK], dt.float32, tag="acc")
                            ps_d = apool.tile([128, NCHUNK], dt.float32, tag="acc")
                            # self term
                            for kc in range(kch):
                                if lay == 1:
                                    rhs = selfT[:, nsl]
                                else:
                                    rhs = selfT[:, kc, nsl]
                                nc.tensor.matmul(ps_v[:], w_ap(lay, "v", kc, mt), rhs,
                                                 start=(kc == 0), stop=(kc == kch - 1))
                            # neighbor-sum terms (single pass over summed G)
                            for term, pst in (("s", ps_s), ("d", ps_d)):
                                tno = 0 if term == "s" else 1
                                for kc in range(kch):
                                    nc.tensor.matmul(
                                        pst[:], w_ap(lay, term, kc, mt),
                                        GS[tno][:, kc, nsl],
                                        start=(kc == 0), stop=(kc == kch - 1))
                            # combine: z = relu(self + s*scl_s + d*scl_d)
                            t1 = tpool.tile([128, NCHUNK], dt.float32, tag="t1")
                            t2 = tpool.tile([128, NCHUNK], dt.float32, tag="t2")
                            t3 = tpool.tile([128, NCHUNK], dt.float32, tag="t3")
                            t4 = tpool.tile([128, NCHUNK], dt.float32, tag="t4")
                            nc.vector.tensor_tensor(t1[:], ps_s[:], scl_ap(p, 0, nch),
                                                    mybir.AluOpType.mult)
                            nc.vector.tensor_tensor(t2[:], ps_d[:], scl_ap(p, 1, nch),
                                                    mybir.AluOpType.mult)
                            nc.vector.scalar_tensor_tensor(
                                t3[:], ps_v[:], 1.0, t1[:],
                                mybir.AluOpType.mult, mybir.AluOpType.add)
                            nc.vector.tensor_tensor(t4[:], t3[:], t2[:],
                                                    mybir.AluOpType.add)
                            nc.scalar.activation(zT[:, mt, nsl], t4[:],
                                                 mybir.ActivationFunctionType.Relu)
                    zT_prev[p] = zT

                    # ---- produce row-major z (AG input / residue rhs) ----
                    if lay < 3:
                        for mt in range(mt_n):
                            zr = zrpool.tile([128, 6, 128], dt.bfloat16, tag="zrow")
                            for ac in range(6):
                                trp = trpool.tile([128, 128], dt.bfloat16, tag="tr")
                                nc.tensor.transpose(
                                    trp[:], zT[:, mt, ac * 128:(ac + 1) * 128], ident[:])
                                nc.any.tensor_copy(zr[:, ac, :], trp[:])
                            bview = zbounce[(lay, p)][:].rearrange(
                                "(ac q) f -> q ac f", q=128)
                            nc.scalar.dma_start(
                                bview[:, :, mt * 128:(mt + 1) * 128], zr[:])
                        nc.gpsimd.collective_compute(
                            "AllGather", mybir.AluOpType.bypass,
                            replica_groups=rg,
                            ins=[zbounce[(lay, p)].opt()],
                            outs=[tabs[lay][p].opt()])
                    else:
                        z3r = zrpool.tile([128, 6, 4, 128], dt.bfloat16, tag="z3row")
                        for mt in range(mt_n):
                            for ac in range(6):
                                trp = trpool.tile([128, 128], dt.bfloat16, tag="tr")
                                nc.tensor.transpose(
                                    trp[:], zT[:, mt, ac * 128:(ac + 1) * 128], ident[:])
                                nc.any.tensor_copy(z3r[:, ac, mt, :], trp[:])
                        z3row[p] = z3r

            # ================= residue partial sums + AllReduce ==========
            for p in range(2):
                for rt in range(RT):
                    ps_r = apool.tile([128, F3], dt.float32, tag="acc")
                    for ac in range(6):
                        nc.tensor.matmul(
                            ps_r[:], s_sb[p][:, ac, rt * 128:(rt + 1) * 128],
                            z3row[p][:, ac, :, :],
                            start=(ac == 0), stop=(ac == 5))
                    rp = tpool.tile([128, F3], dt.bfloat16, tag="rpart")
                    nc.vector.tensor_copy(rp[:], ps_r[:])
                    bview = rbounce_in[p][:].rearrange("(rt q) f -> q rt f", q=128)
                    nc.scalar.dma_start(bview[:, rt, :], rp[:])
                nc.gpsimd.collective_compute(
                    "AllReduce", mybir.AluOpType.add, replica_groups=rg,
                    ins=[rbounce_in[p].opt()], outs=[rbounce_out[p].opt()])

            # ================= r scale, u/v, h, output ===================
            uv = [rhpool.tile([128, RT], dt.float32, tag=f"uv{p}", name=f"uv{p}") for p in range(2)]
            for p in range(2):
                rld16 = rhpool.tile([128, RT, F3], dt.bfloat16, tag=f"rld16{p}",
                                    name=f"rld16{p}")
                rld = rhpool.tile([128, RT, F3], dt.float32, tag=f"rld{p}")
                bview = rbounce_out[p][:].rearrange("(rt q) f -> q rt f", q=128)
                nc.scalar.dma_start(rld16[:], bview[:, :, :])
                for rt in range(RT):
                    nc.vector.tensor_scalar_mul(
                        rld[:, rt, :], rld16[:, rt, :], ricnt[:, rt:rt + 1])
                    junk = tpool.tile([128, F3], dt.float32, tag="rpart", name="junk")
                    nc.vector.scalar_tensor_tensor(
                        junk[:], rld[:, rt, :], 1.0, wab[:, p, :],
                        mybir.AluOpType.mult, mybir.AluOpType.mult,
                        accum_out=uv[p][:, rt:rt + 1])

            # v (protein 2 accumulators) -> rows [1, 128] each, then bcast
            vr = [rhpool.tile([1, 128], dt.float32, tag=f"vr{rt}", name=f"vr{rt}")
                  for rt in range(RT)]
            for rt in range(RT):
                ps_vt = apool.tile([1, 128], dt.float32, tag="acc", name="ps_vt")
                nc.tensor.transpose(ps_vt[:], uv[1][:, rt:rt + 1], ident32[:])
                nc.vector.tensor_copy(vr[rt][:], ps_vt[:])
            ps_vb = apool.tile([128, RPAD], dt.float32, tag="acc")
            for rt in range(RT):
                nc.tensor.matmul(ps_vb[:, rt * 128:(rt + 1) * 128], ones1[:],
                                 vr[rt][:], start=True, stop=True)

            for rt in range(RT):
                hrow = tpool.tile([128, RPAD], dt.float32, tag="hrow", name="hrow")
                outz = tpool.tile([128, NRES], dt.float32, tag="outz", name="outz")
                nc.vector.tensor_scalar(
                    hrow[:], ps_vb[:], uv[0][:, rt:rt + 1], beff[:, 0:1],
                    mybir.AluOpType.add, mybir.AluOpType.add)
                nc.vector.tensor_tensor(
                    outz[:], hrow[:, 0:NRES], hrow[:, 0:NRES],
                    mybir.AluOpType.subtract)
                nc.sync.dma_start(
                    hdbg_d[rt * 128:(rt + 1) * 128, :], hrow[:])
                if rt < 2:
                    oview = out_d[rt * 128 * NRES:(rt + 1) * 128 * NRES, :].rearrange(
                        "(q j) o -> q (j o)", j=NRES)
                    nc.sync.dma_start(oview, outz[:, :])
                else:
                    oview = out_d[2 * 128 * NRES:NPAIR, :].rearrange(
                        "(q j) o -> q (j o)", j=NRES)
                    nc.sync.dma_start(oview, outz[0:NRES - 256, :])

    nc.compile()
    return nc


# ===========================================================================
# host-side input preparation
# ===========================================================================
def _remap_ids(v):
    """old atom id -> padded-shard id; negatives -> ZROW (a zero row)."""
    v = np.asarray(v)
    neg = v < 0
    w = (v // SH_REAL) * SH + (v % SH_REAL)
    w = np.where(neg, ZROW, w)
    return w.astype(np.int16)


def _wrap_idx(flat):
    """[NIDX] int16 -> [128, NIDX//16] wrapped (j at [j%16, j//16]), x8 rep."""
    w16 = flat.reshape(-1, 16).T.copy()        # [16, NIDX//16]
    return np.tile(w16, (8, 1)).astype(np.int16)


def _prep_in_maps(inputs):
    f32 = np.float32
    A = [np.asarray(inputs["atoms1"], f32), np.asarray(inputs["atoms2"], f32)]
    R = [np.asarray(inputs["residues1"], f32), np.asarray(inputs["residues2"], f32)]
    SN = [np.asarray(inputs["same_neigh1"]).astype(np.int64),
          np.asarray(inputs["same_neigh2"]).astype(np.int64)]
    DN = [np.asarray(inputs["diff_neigh1"]).astype(np.int64),
          np.asarray(inputs["diff_neigh2"]).astype(np.int64)]
    RID = [np.asarray(inputs["res_ids1"]).astype(np.int64),
           np.asarray(inputs["res_ids2"]).astype(np.int64)]
    W = {k: np.asarray(inputs[k], f32) for k in
         ["Wv", "Wr", "Wsr1", "Wdr1", "Wsv2", "Wsr2", "Wdr2", "Wsv3", "Wsr3",
          "Wdr3", "fcw1", "fcb1", "fcw2", "fcb2", "fcw3", "fcb3"]}

    # ---- shared (core-independent) inputs -------------------------------
    shared = {}
    for p in range(2):
        z0 = np.zeros((NTOT, F0), f32)
        feats = np.concatenate([A[p], R[p]], 1)          # [6000, 35]
        fr = feats.reshape(NCORES, SH_REAL, NA + NR)
        z0r = z0.reshape(NCORES, SH, F0)
        z0r[:, :SH_REAL, :NA + NR] = fr
        shared[f"t0_{p}"] = z0.astype(bf16)

    def padw(w, rows=128):
        out = np.zeros((rows, w.shape[1]), f32)
        out[:w.shape[0]] = w
        return out

    shared["w0v"] = np.concatenate([W["Wv"], W["Wr"], np.zeros((128 - 35, F1), f32)],
                                   0).astype(bf16)
    shared["w1s"] = padw(W["Wsr1"]).astype(bf16)
    shared["w1d"] = padw(W["Wdr1"]).astype(bf16)
    shared["w2v"] = W["Wsv2"].astype(bf16)
    shared["w2s"] = W["Wsr2"].astype(bf16)
    shared["w2d"] = W["Wdr2"].astype(bf16)
    for nm, k in [("w3v", "Wsv3"), ("w3s", "Wsr3"), ("w3d", "Wdr3")]:
        shared[nm] = W[k].reshape(2, 128, F3).transpose(1, 0, 2).astype(bf16).copy()

    # collapsed FC
    w_eff = (W["fcw1"].astype(np.float64) @ W["fcw2"].astype(np.float64)
             @ W["fcw3"].astype(np.float64)).astype(f32)          # [1024, 1]
    b_eff = float((W["fcb1"].astype(np.float64) @ W["fcw2"].astype(np.float64)
                   @ W["fcw3"].astype(np.float64)
                   + W["fcb2"].astype(np.float64) @ W["fcw3"].astype(np.float64)
                   + W["fcb3"].astype(np.float64)).item())
    wab = np.zeros((128, 2, F3), f32)
    wab[:, 0, :] = np.tile(w_eff[:F3, 0], (128, 1))
    wab[:, 1, :] = np.tile(w_eff[F3:, 0], (128, 1))
    shared["wab"] = wab
    shared["beff"] = np.full((128, 1), b_eff, f32)

    # residue inverse counts (pad residues -> 0 so they contribute nothing)
    ricnt = np.zeros((RPAD,), f32)
    cnt1 = np.bincount(RID[0], minlength=NRES)[:NRES].astype(f32)
    cnt2 = np.bincount(RID[1], minlength=NRES)[:NRES].astype(f32)
    if not np.allclose(cnt1, cnt2):
        # per-protein counts differ; device uses one ricnt tile -> must match.
        # Fall back: counts folded into S instead (weighted membership).
        use_weighted_s = True
    else:
        use_weighted_s = False
    with np.errstate(divide="ignore"):
        inv1 = np.where(cnt1 > 0, 1.0 / np.maximum(cnt1, 1e-30), np.inf).astype(f32)
    ricnt[:NRES] = inv1 if not use_weighted_s else 1.0
    shared["ricnt"] = ricnt.reshape(RT, 128).T.copy()

    # ---- per-core inputs ------------------------------------------------
    in_maps = []
    nbr_cnt = {}
    idx_flat = {}
    for p in range(2):
        for t, NB in ((0, SN[p]), (1, DN[p])):
            cnt = np.maximum((NB >= 0).sum(1), 1).astype(f32)      # [6000]
            nbr_cnt[(p, t)] = cnt
            idx_flat[(p, t)] = _remap_ids(NB)                      # [6000, K]

    for c in range(NCORES):
        m = dict(shared)
        for p in range(2):
            m[f"z0t_{p}"] = np.ascontiguousarray(
                shared[f"t0_{p}"][c * SH:(c + 1) * SH, :].T)
            # membership matrix S [768, RPAD] -> [128, 6, RPAD]
            S = np.zeros((SH, RPAD), f32)
            rid_c = RID[p][c * SH_REAL:(c + 1) * SH_REAL]
            S[np.arange(SH_REAL), rid_c] = 1.0
            if use_weighted_s:
                cnts = np.bincount(RID[p], minlength=NRES)[:NRES].astype(f32)
                with np.errstate(divide="ignore"):
                    invp = np.where(cnts > 0, 1.0 / np.maximum(cnts, 1e-30), np.inf)
                S[np.arange(SH_REAL), rid_c] = invp[rid_c]
            m[f"s_{p}"] = S.reshape(6, 128, RPAD).transpose(1, 0, 2).astype(bf16).copy()
            for t in range(2):
                lists = idx_flat[(p, t)][c * SH_REAL:(c + 1) * SH_REAL]  # [750, K]
                full = np.full((SH, KNEI), ZROW, np.int16)
                full[:SH_REAL] = lists
                m[f"idx_{p}{t}"] = _wrap_idx(full.T.reshape(-1))  # k-major
        scl = np.zeros((128, 4, SH), f32)  # cast to bf16 below
        for p in range(2):
            for t in range(2):
                inv = np.zeros((SH,), f32)
                inv[:SH_REAL] = 1.0 / nbr_cnt[(p, t)][c * SH_REAL:(c + 1) * SH_REAL]
                scl[:, p * 2 + t, :] = np.tile(inv, (128, 1))
        m["scl"] = scl.astype(bf16)
        in_maps.append(m)
    return in_maps


# ===========================================================================
def kernel(**inputs) -> np.ndarray:
    global _PROG
    from concourse.bass_utils import run_bass_kernel_spmd

    if _PROG is None:
        _PROG = _build_program()
    in_maps = _prep_in_maps(inputs)
    res = run_bass_kernel_spmd(_PROG, in_maps, core_ids=list(range(NCORES)))
    out = np.asarray(res.results[0]["out"], np.float32).reshape(NPAIR, 1)
    return out


if __name__ == "__main__":
    rng = np.random.default_rng(0)
    print("kernel.py loaded")

